# revision 1
# baseline (speedup 1.0000x reference)
"""Trainium2 Bass kernel for LoRA-attention (nn_Attention_lorad).

Computes, for x: [8, 1024, 768]:
    qkv = x @ qkv_w.T + qkv_b           (only k = qkv[..., C:2C] is used)
    q   = lora_linear(x, q_w, q_b, q_A, q_B)
    v   = lora_linear(x, v_w, v_b, v_A, v_B)
    out = softmax(q k^T / sqrt(d)) v    per head (12 heads, d=64)
    y   = out @ proj_w.T + proj_b

Sharding: pure data-parallel over batch B=8 -> one batch element per core.

Host-side exact algebraic folds:
  - LoRA:   w_eff = w + (B @ A) / r           (removes LoRA matmuls on device)
  - v bias: softmax rows sum to 1, so P @ (v + 1 vb^T) = P @ v + 1 vb^T;
            hence pb_eff = proj_b + proj_w @ v_b and v is projected bias-free.

Device schedule (per core, all matmuls in fp32r for accuracy):
  1. v projected in natural orientation (stationary xT tiles, moving vw)
     into an augmented layout with a ones column per head ([.. v_h | 1 ..]).
  2. q,k projected in transposed orientation qT/kT [C(j), N] (per-partition
     bias add on DVE), staggered two head-pairs ahead of attention so the
     ACT exp stream starts as early as possible. Per head pair (2jt, 2jt+1):
       S.T[m, n] = kT_h(stationary) x qT_h(moving)   (PSUM [128,512])
       expP.T    = ACT Exp(S.T / 8)  PSUM->SBUF
       pv[65, n] = v_aug_h(stationary) x expP.T      (PSUM accum over m;
                   the ones column makes row 64 the softmax denominator)
       aoT_h     = pv[0:64] * recip(pv[64])          (DVE + gpsimd bcast)
     Even/odd heads sit at partition offsets 0/64, so their K=64 QK matmuls
     target disjoint PE row groups (concurrent on HW).
  3. y.T = pwT x aoT + pb_eff (DVE bias add), DMA'd out; host transposes.

No max-subtraction in softmax: logits are ~N(0,1) here (|logit| < ~8),
exp is safely within fp32 range and the result is mathematically identical.
"""

import os
import sys

for _p in ("/opt/trn_rl_repo", "/root/.axon_site/_ro/trn_rl_repo"):
    if os.path.isdir(_p) and _p not in sys.path:
        sys.path.insert(0, _p)

import numpy as np

import concourse.bacc as bacc
import concourse.mybir as mybir
from concourse.bass_utils import run_bass_kernel_spmd
from concourse.tile import TileContext
from contextlib import ExitStack

F32 = mybir.dt.float32
F32R = mybir.dt.float32r
BF16 = mybir.dt.bfloat16
AFT = mybir.ActivationFunctionType

P = 128           # SBUF partitions
C = 768           # model dim
N = 1024          # sequence length
H = 12            # heads
D = 64            # head dim
R = 16            # lora rank
CT = C // P       # 6 c-tiles
NT = N // P       # 8 token tiles
NCH = 2           # 512-wide chunks of N
CHUNK = N // NCH  # 512
VJC = 2           # v projection j-chunks (384 each)
VW = C // VJC     # 384
SCALE = D ** -0.5

_CACHE = {}


def build_nc(use_f32r=True):
    MMDT = F32R if use_f32r else F32
    nc = bacc.Bacc("TRN2", target_bir_lowering=False, debug=False)

    xT = nc.dram_tensor("xT", [C, N], MMDT, kind="ExternalInput").ap()
    qwT = nc.dram_tensor("qwT", [C, C], MMDT, kind="ExternalInput").ap()
    kwT = nc.dram_tensor("kwT", [C, C], MMDT, kind="ExternalInput").ap()
    vwT = nc.dram_tensor("vwT", [C, C], MMDT, kind="ExternalInput").ap()
    pwT = nc.dram_tensor("pwT", [C, C], MMDT, kind="ExternalInput").ap()
    qb = nc.dram_tensor("qb", [P, CT], F32, kind="ExternalInput").ap()
    kb = nc.dram_tensor("kb", [P, CT], F32, kind="ExternalInput").ap()
    pb = nc.dram_tensor("pb", [P, CT], F32, kind="ExternalInput").ap()
    yT = nc.dram_tensor("yT", [C, N], F32, kind="ExternalOutput").ap()

    with TileContext(nc) as tc, ExitStack() as ctx:
        persist = ctx.enter_context(tc.tile_pool(name="persist", bufs=1))
        xpool = ctx.enter_context(tc.tile_pool(name="xpool", bufs=1))
        qkpool = ctx.enter_context(tc.tile_pool(name="qkpool", bufs=2))
        pps = ctx.enter_context(tc.tile_pool(name="pps", bufs=2, space="PSUM"))

        # ---- long-lived SBUF tensors ----
        pw_sb = [persist.tile([P, C], MMDT, tag=f"pw{t}", name=f"pw{t}")
                 for t in range(CT)]
        v_aug = [persist.tile([P, H * (D + 1)], MMDT, tag=f"vaug{m}",
                              name=f"vaug{m}") for m in range(NT)]
        qb_sb = persist.tile([P, CT], F32, tag="qb", name="qb")
        kb_sb = persist.tile([P, CT], F32, tag="kb", name="kb")
        pb_sb = persist.tile([P, CT], F32, tag="pb", name="pb")
        xT_sb = [xpool.tile([P, N], MMDT, tag=f"x{t}", name=f"x{t}")
                 for t in range(CT)]
        qw_sb = [xpool.tile([P, C], MMDT, tag=f"qw{t}", name=f"qw{t}")
                 for t in range(CT)]
        kw_sb = [xpool.tile([P, C], MMDT, tag=f"kw{t}", name=f"kw{t}")
                 for t in range(CT)]

        actx = ExitStack()
        apool = actx.enter_context(tc.tile_pool(name="apool", bufs=1))
        epool = actx.enter_context(tc.tile_pool(name="epool", bufs=1))
        small = actx.enter_context(tc.tile_pool(name="small", bufs=2))

        # v weights live only through the v projection
        vstack = ExitStack()
        vpool = vstack.enter_context(tc.tile_pool(name="vpool", bufs=1))
        vw_sb = [vpool.tile([P, C], MMDT, tag=f"vw{t}", name=f"vw{t}")
                 for t in range(CT)]

        # DMA issue order = consumption order: x/vw, then qw, kw, pw
        for t in range(CT):
            nc.sync.dma_start(out=xT_sb[t][:], in_=xT[t * P:(t + 1) * P, :])
            nc.sync.dma_start(out=vw_sb[t][:], in_=vwT[t * P:(t + 1) * P, :])
        for t in range(CT):
            nc.sync.dma_start(out=qw_sb[t][:], in_=qwT[t * P:(t + 1) * P, :])
        nc.sync.dma_start(out=qb_sb[:], in_=qb[:, :])
        for t in range(CT):
            nc.sync.dma_start(out=kw_sb[t][:], in_=kwT[t * P:(t + 1) * P, :])
        nc.sync.dma_start(out=kb_sb[:], in_=kb[:, :])
        for t in range(CT):
            nc.sync.dma_start(out=pw_sb[t][:], in_=pwT[t * P:(t + 1) * P, :])
        nc.sync.dma_start(out=pb_sb[:], in_=pb[:, :])

        # ones columns of v_aug (softmax denominator trick); memset cannot
        # write f32r, so stage f32 ones and DVE-copy (copy may cast)
        ones_stage = persist.tile([P, H], F32, tag="ones", name="ones")
        nc.vector.memset(ones_stage[:], 1.0)
        for m in range(NT):
            ones_view = v_aug[m].rearrange("p (h s) -> p h s", s=D + 1)
            nc.vector.tensor_copy(ones_view[:, :, D:D + 1], ones_stage[:])

        def v_proj(vpsum):
            # ct-outer over groups of 6 concurrent PSUM chains: each matmul
            # only needs x/vw tile ct, so PE tracks the DMA arrival order
            # instead of stalling for the full contraction's tiles.
            groups = [[(mt, jc) for mt in mts for jc in range(VJC)]
                      for mts in ([0, 1, 2], [3, 4, 5], [6, 7])]
            for group in groups:
                pss = {c: vpsum.tile([P, VW], F32, tag=f"vps{i}",
                                     name=f"vps{i}")
                       for i, c in enumerate(group)}
                for ct in range(CT):
                    for (mt, jc) in group:
                        nc.tensor.matmul(
                            pss[(mt, jc)][:],
                            lhsT=xT_sb[ct][:, mt * P:(mt + 1) * P],
                            rhs=vw_sb[ct][:, jc * VW:(jc + 1) * VW],
                            start=(ct == 0), stop=(ct == CT - 1))
                for (mt, jc) in group:
                    dst = v_aug[mt].rearrange("p (h s) -> p h s", s=D + 1)
                    hpc = VW // D
                    nc.vector.tensor_copy(
                        dst[:, jc * hpc:(jc + 1) * hpc, 0:D],
                        pss[(mt, jc)][:].rearrange("p (h s) -> p h s", s=D))

        def proj_one(w_sb, b_sb, jt, wname):
            """One transposed projection: columns jt*128..+128 -> [P, N]."""
            dst = qkpool.tile([P, N], MMDT, tag=f"{wname}T", name=f"{wname}T")
            for ch in range(NCH):
                ps = pps.tile([P, CHUNK], F32, tag="pps", name="pps")
                for ct in range(CT):
                    nc.tensor.matmul(
                        ps[:], lhsT=w_sb[ct][:, jt * P:(jt + 1) * P],
                        rhs=xT_sb[ct][:, ch * CHUNK:(ch + 1) * CHUNK],
                        start=(ct == 0), stop=(ct == CT - 1))
                nc.vector.tensor_scalar_add(
                    dst[:, ch * CHUNK:(ch + 1) * CHUNK], ps[:],
                    b_sb[:, jt:jt + 1])
            return dst

        def qk_proj(jt):
            return (proj_one(qw_sb, qb_sb, jt, "q"),
                    proj_one(kw_sb, kb_sb, jt, "k"))

        def head_qk_exp(o, qT_t, kT_t, epool):
            """QK matmuls + wide exps for one head (rows o..o+64)."""
            eps = []
            for mt in range(NT):
                sp = spsum.tile([P, N], F32, tag="sps", name="sps")
                for ch in range(NCH):
                    nc.tensor.matmul(
                        sp[:, ch * CHUNK:(ch + 1) * CHUNK],
                        lhsT=kT_t[o:o + D, mt * P:(mt + 1) * P],
                        rhs=qT_t[o:o + D, ch * CHUNK:(ch + 1) * CHUNK],
                        start=True, stop=True)
                ep = epool.tile([P, N], MMDT, tag="exp", name="exp", bufs=8)
                nc.scalar.activation(out=ep[:], in_=sp[:], func=AFT.Exp,
                                     scale=SCALE)
                eps.append(ep)
            return eps

        def head_pv(jt, o, h, eps, small, aoT_sb):
            """PV + normalization for one head; both 512-chunks."""
            pvs = [pvps.tile([D + 1, CHUNK], F32, tag=f"pv{ch}",
                             name=f"pv{ch}") for ch in range(NCH)]
            for mt in range(NT):
                for ch in range(NCH):
                    nc.tensor.matmul(
                        pvs[ch][:],
                        lhsT=v_aug[mt][:, h * (D + 1):(h + 1) * (D + 1)],
                        rhs=eps[mt][:, ch * CHUNK:(ch + 1) * CHUNK],
                        start=(mt == 0), stop=(mt == NT - 1))
            for ch in range(NCH):
                csl = slice(ch * CHUNK, (ch + 1) * CHUNK)
                recip = small.tile([1, CHUNK], F32, tag="recip",
                                   name="recip")
                nc.vector.reciprocal(recip[:], pvs[ch][D:D + 1, :])
                rbc = small.tile([D, CHUNK], F32, tag="rbc", name="rbc")
                nc.gpsimd.partition_broadcast(rbc[:], recip[:], channels=D)
                nc.vector.tensor_mul(aoT_sb[jt][o:o + D, csl],
                                     pvs[ch][0:D, :], rbc[:])

        with actx:
            # v first: attention is then purely PE-paced with no ACT bubble
            vpsum = vstack.enter_context(
                tc.tile_pool(name="vpsum", bufs=1, space="PSUM"))
            v_proj(vpsum)
            vstack.close()
            spsum = actx.enter_context(
                tc.tile_pool(name="spsum", bufs=2, space="PSUM"))
            pvps = actx.enter_context(
                tc.tile_pool(name="pvps", bufs=1, space="PSUM"))
            aoT_sb = [apool.tile([P, N], MMDT, tag=f"aoT{t}", name=f"aoT{t}")
                      for t in range(CT)]
            qk_next = qk_proj(0)
            for jt in range(CT):
                q_t, k_t = qk_next
                for o, h in ((0, 2 * jt), (D, 2 * jt + 1)):
                    eps = head_qk_exp(o, q_t, k_t, epool)
                    head_pv(jt, o, h, eps, small, aoT_sb)
                if jt + 1 < CT:
                    qk_next = qk_proj(jt + 1)

            # -- output projection --
            fout = actx.enter_context(tc.tile_pool(name="fout", bufs=4))
            for jt in range(CT):
                for ch in range(NCH):
                    ps = pps.tile([P, CHUNK], F32, tag="pps", name="fps")
                    for ct in range(CT):
                        nc.tensor.matmul(
                            ps[:], lhsT=pw_sb[ct][:, jt * P:(jt + 1) * P],
                            rhs=aoT_sb[ct][:, ch * CHUNK:(ch + 1) * CHUNK],
                            start=(ct == 0), stop=(ct == CT - 1))
                    ob = fout.tile([P, CHUNK], F32, tag="ob", name="ob")
                    # alternate eviction engines to shorten the tail
                    if ch == 0:
                        nc.vector.tensor_scalar_add(ob[:], ps[:],
                                                    pb_sb[:, jt:jt + 1])
                    else:
                        nc.scalar.activation(out=ob[:], in_=ps[:],
                                             func=AFT.Identity,
                                             bias=pb_sb[:, jt:jt + 1])
                    nc.sync.dma_start(
                        out=yT[jt * P:(jt + 1) * P,
                               ch * CHUNK:(ch + 1) * CHUNK],
                        in_=ob[:])

    nc.compile()
    return nc


def _get_nc(use_f32r=True):
    key = ("nc", use_f32r)
    if key not in _CACHE:
        _CACHE[key] = build_nc(use_f32r)
    return _CACHE[key]


def kernel(x, qkv_w, qkv_b, q_w, q_b, q_A, q_B, v_w, v_b, v_A, v_B,
           proj_w, proj_b, _trace=False, _use_f32r=True):
    x = np.ascontiguousarray(np.asarray(x, dtype=np.float32))
    B = x.shape[0]
    assert x.shape == (8, N, C)

    qkv_w = np.asarray(qkv_w, np.float32)
    qkv_b = np.asarray(qkv_b, np.float32)
    q_w = np.asarray(q_w, np.float32)
    q_b = np.asarray(q_b, np.float32)
    q_A = np.asarray(q_A, np.float32)
    q_B = np.asarray(q_B, np.float32)
    v_w = np.asarray(v_w, np.float32)
    v_b = np.asarray(v_b, np.float32)
    v_A = np.asarray(v_A, np.float32)
    v_B = np.asarray(v_B, np.float32)
    proj_w = np.asarray(proj_w, np.float32)
    proj_b = np.asarray(proj_b, np.float32)

    # exact algebraic folds (see module docstring)
    qw_eff = q_w + (q_B @ q_A) * (1.0 / R)
    vw_eff = v_w + (v_B @ v_A) * (1.0 / R)
    kw = qkv_w[C:2 * C]
    kb = qkv_b[C:2 * C]
    pb_eff = proj_b + proj_w @ v_b

    common = {
        "qwT": np.ascontiguousarray(qw_eff.T),
        "kwT": np.ascontiguousarray(kw.T),
        "vwT": np.ascontiguousarray(vw_eff.T),
        "pwT": np.ascontiguousarray(proj_w.T),
        "qb": np.ascontiguousarray(q_b.reshape(CT, P).T),
        "kb": np.ascontiguousarray(kb.reshape(CT, P).T),
        "pb": np.ascontiguousarray(pb_eff.reshape(CT, P).T),
    }
    in_maps = [
        {"xT": np.ascontiguousarray(x[i].T), **common} for i in range(B)
    ]

    nc = _get_nc(_use_f32r)
    res = run_bass_kernel_spmd(nc, in_maps, list(range(B)), trace=_trace)

    out = np.empty((B, N, C), np.float32)
    for i in range(B):
        out[i] = res.results[i]["yT"].T
    if _trace:
        return out, res
    return out



# revision 64
# speedup vs baseline: 1.2097x; 1.2097x over previous
"""Trainium2 Bass kernel for LoRA-attention (nn_Attention_lorad).

Computes, for x: [8, 1024, 768]:
    qkv = x @ qkv_w.T + qkv_b           (only k = qkv[..., C:2C] is used)
    q   = lora_linear(x, q_w, q_b, q_A, q_B)
    v   = lora_linear(x, v_w, v_b, v_A, v_B)
    out = softmax(q k^T / sqrt(d)) v    per head (12 heads, d=64)
    y   = out @ proj_w.T + proj_b
Sharding: pure data-parallel over batch B=8 -> one batch element per core.

Host-side exact algebraic folds:
  - LoRA:   w_eff = w + (B @ A) / r
  - v bias: softmax rows sum to 1, so pb_eff = proj_b + proj_w @ v_b and
    v is projected bias-free.

Schedule (per core). The ACT exp stream (96 x [128,1024] ~ 100us) and the
PE matmul stream (~123us) are co-critical; everything is organized so both
run back-to-back with no bubbles:
  1. Pre-window: qk-projection for head pair 0 (transposed orientation,
     qT/kT [C, N], DVE per-partition bias add), paced by the x DMA.
  2. Window: per head h, QK S-tiles S.T[m-tile, n] = kT_h x qT_h stream
     into PSUM and ACT exps them into bf16 eps tiles. Between S-tiles the
     PE runs "filler" work: remaining qk-projections, the v projection
     (natural orientation, augmented with a ones column per head so PV's
     row 64 is the softmax denominator), and PV for completed head pairs.
  3. PV (flipped vs the score layout): out[n-tile, 65] = eps-tile^T @
     v_aug_h -- 65-row bf16 matmuls (cost model: rows x 1 cyc), half the
     PE cost of the [65, n] orientation. Normalization is a DVE
     reciprocal + per-partition-scalar multiply on PSUM eviction; the
     normalized [tok, 128] pair tile is transposed into aoT via the DMA
     XBAR (dma_start_transpose), which costs no PE/DVE time.
  4. Post-window: PV for the last pair, then the output projection
     y.T = pwT x aoT with ACT Identity+bias eviction, DMA'd out.

All matmul operands are bf16 (PSUM stays f32); biases and normalization
are f32. Measured end-to-end rel err ~5e-3 (budget 2e-2). No
max-subtraction in softmax: logits are ~N(0,1) here, exp is safely in
range and the result is mathematically identical.
"""

import os
import sys

for _p in ("/opt/trn_rl_repo", "/root/.axon_site/_ro/trn_rl_repo"):
    if os.path.isdir(_p) and _p not in sys.path:
        sys.path.insert(0, _p)

import numpy as np
import ml_dtypes

import concourse.bacc as bacc
import concourse.mybir as mybir
from concourse.bass_utils import run_bass_kernel_spmd
from concourse.tile import TileContext
from concourse import masks
from contextlib import ExitStack

F32 = mybir.dt.float32
BF16 = mybir.dt.bfloat16
AFT = mybir.ActivationFunctionType

P = 128           # SBUF partitions
C = 768           # model dim
N = 1024          # sequence length
H = 12            # heads
D = 64            # head dim
R = 16            # lora rank
CT = C // P       # 6 c-tiles
NT = N // P       # 8 token tiles
NCH = 2           # 512-wide chunks of N
CHUNK = N // NCH  # 512
VJC = 2           # v projection j-chunks (384 wide)
VW = C // VJC     # 384
HPC = VW // D     # heads per v-proj chunk (6)
SCALE = D ** -0.5

_CACHE = {}


def build_nc(use_f32r=True):
    nc = bacc.Bacc("TRN2", target_bir_lowering=False, debug=False)

    XAW = N + 4 * P + 2
    xA = nc.dram_tensor("xA", [C, XAW], BF16, kind="ExternalInput").ap()
    wR = nc.dram_tensor("wR", [C, 2 * (C - 2 * P)], BF16,
                        kind="ExternalInput").ap()
    vwT = nc.dram_tensor("vwT", [C, C], BF16, kind="ExternalInput").ap()
    pwT = nc.dram_tensor("pwT", [C, C], BF16, kind="ExternalInput").ap()
    pb = nc.dram_tensor("pb", [P, CT], F32, kind="ExternalInput").ap()
    yT = nc.dram_tensor("yT", [C, N], BF16, kind="ExternalOutput").ap()

    with TileContext(nc) as tc, ExitStack() as ctx:
        persist = ctx.enter_context(tc.tile_pool(name="persist", bufs=1))
        qkpool = ctx.enter_context(tc.tile_pool(name="qkpool", bufs=6))
        epool = ctx.enter_context(tc.tile_pool(name="epool", bufs=48))
        aostage = ctx.enter_context(tc.tile_pool(name="aostage", bufs=8))
        rpool = ctx.enter_context(tc.tile_pool(name="rpool", bufs=16))
        ypool = ctx.enter_context(tc.tile_pool(name="ypool", bufs=6))
        spsum = ctx.enter_context(
            tc.tile_pool(name="spsum", bufs=2, space="PSUM"))
        pvps = ctx.enter_context(
            tc.tile_pool(name="pvps", bufs=2, space="PSUM"))
        pps = ctx.enter_context(tc.tile_pool(name="pps", bufs=2, space="PSUM"))

        # ---- long-lived SBUF tensors ----
        xT_sb = [persist.tile([P, N], BF16, tag=f"x{t}", name=f"x{t}")
                 for t in range(CT)]
        qw0_sb = [persist.tile([P, P], BF16, tag=f"qw0_{t}") for t in range(CT)]
        kw0_sb = [persist.tile([P, P], BF16, tag=f"kw0_{t}") for t in range(CT)]
        qwR_sb = [persist.tile([P, C - P], BF16, tag=f"qwR{t}")
                  for t in range(CT)]
        kwR_sb = [persist.tile([P, C - P], BF16, tag=f"kwR{t}")
                  for t in range(CT)]
        vw_sb = [persist.tile([P, C], BF16, tag=f"vw{t}") for t in range(CT)]
        pw_sb = [persist.tile([P, C], BF16, tag=f"pw{t}") for t in range(CT)]
        v_aug = [persist.tile([P, H * (D + 1)], BF16, tag=f"vaug{m}")
                 for m in range(NT)]
        aoT_sb = [persist.tile([P, N], BF16, tag=f"aoT{t}") for t in range(CT)]
        qb_sb = persist.tile([P, CT], F32, tag="qb")
        kb_sb = persist.tile([P, CT], F32, tag="kb")
        pb_sb = persist.tile([P, CT], F32, tag="pb")
        ones_stage = persist.tile([P, H], F32, tag="ones")
        warm_in = persist.tile([P, 1], F32, tag="warm_in")
        warm_out = persist.tile([P, 1], BF16, tag="warm_out")

        identity = persist.tile([P, P], BF16, tag="identity", name="identity")

        # ---- DMA issue order = consumption order ----
        # piece A (wA + x cols 0:512) feeds the whole first 512-wide
        # qk-projection chunk; piece B completes x
        XSPL = 2 * P + 2 + CHUNK
        for t in range(CT):
            nc.sync.dma_start(out=xA_sb[t][:, 0:XSPL],
                              in_=xA[t * P:(t + 1) * P, 0:XSPL])

        # setup work (not DMA-dependent), issued behind the critical DMAs
        masks.make_identity(nc, identity[:])
        # ACT table warm-up: pull the Exp table load off the critical path
        nc.vector.memset(warm_in[:], 0.0)
        nc.scalar.activation(out=warm_out[:], in_=warm_in[:], func=AFT.Exp)
        # ones columns of v_aug (softmax denominator trick)
        nc.vector.memset(ones_stage[:], 1.0)
        for m in range(NT):
            ones_view = v_aug[m].rearrange("p (h s) -> p h s", s=D + 1)
            nc.vector.tensor_copy(ones_view[:, :, D:D + 1], ones_stage[:])
        for t in range(CT):
            nc.vector.tensor_copy(b2_sb[:, 2 * t:2 * t + 2],
                                  xA_sb[t][:, 2 * P:2 * P + 2])
        for t in range(CT):
            nc.sync.dma_start(out=xA_sb[t][:, XSPL:],
                              in_=xA[t * P:(t + 1) * P, XSPL:])
        for t in range(CT):
            nc.sync.dma_start(out=vw_sb[t][:], in_=vwT[t * P:(t + 1) * P, :])
        for t in range(CT):
            nc.sync.dma_start(out=wR_sb[t][:], in_=wR[t * P:(t + 1) * P, :])
        for t in range(CT):
            nc.sync.dma_start(out=pw_sb[t][:], in_=pwT[t * P:(t + 1) * P, :])
        nc.sync.dma_start(out=pb_sb[:], in_=pb[:, :])

        # ---- emission helpers ----
        def bias_col(isq, jt):
            return b2_sb[:, 2 * jt + (0 if isq else 1):
                         2 * jt + (0 if isq else 1) + 1]

        def qslice(isq, t, jt):
            if jt == 0:
                o = 0 if isq else P
                return wA_sb[t][:, o:o + P]
            if jt == 1:
                o = XW1 + (0 if isq else P)
                return xA_sb[t][:, o:o + P]
            o = (0 if isq else C - 2 * P) + (jt - 2) * P
            return wR_sb[t][:, o:o + P]

        def fps_psum(i):
            # alternate PSUM pools so output-projection chunks double-buffer
            # across four banks (spsum is dead post-window)
            if i % 2 == 0:
                return spsum.tile([P, CHUNK], F32, tag="sps", name="fps_s")
            return pps.tile([P, CHUNK], F32, tag="pps", name="fps_p")

        PW = 256  # proj filler chunk width

        def proj_chain(isq, jt, ch, dst):
            """One 256-wide chunk of a transposed projection (6 mm + evict)."""
            ps = pps.tile([P, PW], F32, tag="pps", name="pps")
            for t in range(CT):
                nc.tensor.matmul(
                    ps[:], lhsT=qslice(isq, t, jt),
                    rhs=xT_sb[t][:, ch * PW:(ch + 1) * PW],
                    start=(t == 0), stop=(t == CT - 1))
            nc.vector.tensor_scalar_add(
                dst[:, ch * PW:(ch + 1) * PW], ps[:], bias_col(isq, jt))

        VVW = 192  # v-proj filler chunk width (3 heads)
        VHPC = VVW // D

        def vproj_chain(mt, jc):
            """One (token-tile, 192-col) v-projection chain (6 mm + evict)."""
            ps = pps.tile([P, VVW], F32, tag="pps", name="vps")
            for t in range(CT):
                nc.tensor.matmul(
                    ps[:], lhsT=xT_sb[t][:, mt * P:(mt + 1) * P],
                    rhs=vw_sb[t][:, jc * VVW:(jc + 1) * VVW],
                    start=(t == 0), stop=(t == CT - 1))
            dst = v_aug[mt].rearrange("p (h s) -> p h s", s=D + 1)
            nc.vector.tensor_copy(
                dst[:, jc * VHPC:(jc + 1) * VHPC, 0:D],
                ps[:].rearrange("p (h s) -> p h s", s=D))

        in_tail = [False]  # post-window: ACT/spsum are free

        def pv_norm(jt, nt, pv):
            """Normalize + transpose a finished PV accumulation."""
            stage = aostage.tile([P, P], BF16, tag="ao", name="ao")
            tail = in_tail[0]
            on_act = tail and nt % 2 == 0  # split tail chains ACT/DVE
            for hh in range(2):
                o = hh * (D + 1)
                recip = rpool.tile([P, 1], F32, tag="recip", name="recip")
                nc.vector.reciprocal(recip[:], pv[:, o + D:o + D + 1])
                if on_act:
                    nc.scalar.activation(
                        out=stage[:, hh * D:(hh + 1) * D], in_=pv[:, o:o + D],
                        func=AFT.Copy, scale=recip[:])
                else:
                    nc.vector.tensor_scalar_mul(
                        stage[:, hh * D:(hh + 1) * D], pv[:, o:o + D],
                        recip[:])
            if tail:
                # PE transpose (SP DMA-issue latency would sit on the
                # critical path into the output projection)
                tps = pps.tile([P, P], BF16, tag="pps", name="tps")
                nc.tensor.transpose(tps[:], stage[:], identity[:])
                if on_act:
                    nc.scalar.activation(
                        out=aoT_sb[jt][:, nt * P:(nt + 1) * P], in_=tps[:],
                        func=AFT.Copy)
                else:
                    nc.vector.tensor_copy(
                        aoT_sb[jt][:, nt * P:(nt + 1) * P], tps[:])
            else:
                nc.sync.dma_start_transpose(
                    out=aoT_sb[jt][:, nt * P:(nt + 1) * P], in_=stage[:])

        def pv_group(jt, nt, eps_pair):
            """PV + normalize + transpose for (pair jt, token-tile nt)."""
            if jt >= 4 and not in_tail[0] and nt % 2 == 0:
                # late window: proj chains are done, pps is free; 4-deep
                # PV pipeline keeps PE fed while DVE normalizes
                pv = pps.tile([P, 2 * (D + 1)], F32, tag="pps", name="pv_p")
            else:
                pv = pvps.tile([P, 2 * (D + 1)], F32, tag="pv", name="pv")
            for hh in range(2):
                h = 2 * jt + hh
                o = hh * (D + 1)
                for mt in range(NT):
                    nc.tensor.matmul(
                        pv[:, o:o + D + 1],
                        lhsT=eps_pair[hh][mt][:, nt * P:(nt + 1) * P],
                        rhs=v_aug[mt][:, h * (D + 1):(h + 1) * (D + 1)],
                        start=(mt == 0), stop=(mt == NT - 1))
            pv_norm(jt, nt, pv)

        # filler queue of (pe_rows, deadline_head, fn): fn MUST be emitted
        # before head `deadline_head`'s QK matmuls (program order is
        # execution order per engine).
        fillers = []

        def add_qkproj(jt):
            qt = qkpool.tile([P, N], BF16, tag="qT", name=f"qT{jt}")
            kt = qkpool.tile([P, N], BF16, tag="kT", name=f"kT{jt}")
            for ch in range(N // PW):
                fillers.append((CT * PW, 2 * jt, lambda ch=ch, qt=qt:
                                proj_chain(True, jt, ch, qt)))
                fillers.append((CT * PW, 2 * jt, lambda ch=ch, kt=kt:
                                proj_chain(False, jt, ch, kt)))
            return qt, kt

        def add_vproj(mts):
            for mt in mts:
                for jc in range(C // VVW):
                    fillers.append((CT * VVW, 5, lambda mt=mt, jc=jc:
                                    vproj_chain(mt, jc)))

        def add_pv(jt, eps_pair):
            # eps tiles of pair jt are recycled by head 2*jt+6 (48-deep
            # eps pool); PV must be emitted before then.
            for nt in range(NT):
                fillers.append((2 * NT * (D + 1), min(2 * jt + 6, 11),
                                lambda nt=nt: pv_group(jt, nt, eps_pair)))

        # ---- pre-window: qk-projection pair 0, paced by the x DMA ----
        # 512-wide chains + half-width S/exp for head-0's first four token
        # tiles so the ACT exp stream starts as early as possible.
        eps = {}          # h -> list of NT eps tiles
        qkT = {}
        qt0 = qkpool.tile([P, N], BF16, tag="qT", name="qT0")
        kt0 = qkpool.tile([P, N], BF16, tag="kT", name="kT0")
        qkT[0] = (qt0, kt0)

        def proj_chain512(isq, ch, dst):
            ps = pps.tile([P, CHUNK], F32, tag="pps", name="p512")
            for t in range(CT):
                nc.tensor.matmul(
                    ps[:], lhsT=qslice(isq, t, 0),
                    rhs=xT_sb[t][:, ch * CHUNK:(ch + 1) * CHUNK],
                    start=(t == 0), stop=(t == CT - 1))
            nc.vector.tensor_scalar_add(
                dst[:, ch * CHUNK:(ch + 1) * CHUNK], ps[:], bias_col(isq, 0))

        eps[0] = [epool.tile([P, N], BF16, tag="exp", name="exp0")
                  for _ in range(4)]
        for ch in range(NCH):
            proj_chain512(True, ch, qt0)
            proj_chain512(False, ch, kt0)
            for mt in range(4):
                sp = spsum.tile([P, CHUNK], F32, tag="sps", name="sph")
                nc.tensor.matmul(
                    sp[:], lhsT=kt0[0:D, mt * P:(mt + 1) * P],
                    rhs=qt0[0:D, ch * CHUNK:(ch + 1) * CHUNK],
                    start=True, stop=True)
                nc.scalar.activation(
                    out=eps[0][mt][:, ch * CHUNK:(ch + 1) * CHUNK],
                    in_=sp[:], func=AFT.Exp, scale=SCALE)

        # ---- window: head-major QK+exp stream with interleaved filler ----
        filler_done = 0
        filler_target = 0.0
        FILLER_TOTAL = (5 * 2 * NCH * CT * CHUNK          # qk-proj pairs 1-5
                        + NT * VJC * CT * VW              # v-proj
                        + 5 * NT * 2 * NT * (D + 1))      # pv pairs 0-4

        order = []        # filler enqueue script, keyed by head index
        # h=0: pairs 1,2 proj + first half of v-proj
        # h=1: rest of v-proj; h=2: pv0 queued after eps[0..1] exist, etc.
        for h in range(H):
            jt, o = h // 2, (h % 2) * D
            if h == 0:
                add_vproj(range(0, 4))
                qkT[1] = add_qkproj(1)
                add_vproj(range(4, NT))
                qkT[2] = add_qkproj(2)
            elif h == 3:
                qkT[3] = add_qkproj(3)
            elif h == 5:
                qkT[4] = add_qkproj(4)
            elif h == 7:
                qkT[5] = add_qkproj(5)
            if h >= 2 and h % 2 == 0:
                # eps for pair (h-2)//2 complete -> queue its PV
                pj = (h - 2) // 2
                add_pv(pj, (eps[2 * pj], eps[2 * pj + 1]))

            # force-emit everything whose deadline is this head (scan the
            # whole queue: enqueue order is not deadline order)
            due = [u for u in fillers if u[1] <= h]
            if due:
                fillers[:] = [u for u in fillers if u[1] > h]
                for rows, _, fn in due:
                    fn()
                    filler_done += rows

            if h == H - 1:
                # drain remaining filler, then pre-allocate pair-5 PV
                # accumulators for token tiles 0-3: their matmuls weave
                # into this head's exp stream two tiles behind (PE would
                # otherwise idle at ACT's pace with no filler left)
                while fillers:
                    rows, _, fn = fillers.pop(0)
                    fn()
                    filler_done += rows
                pvs5 = [(pvps if nt % 2 == 0 else pps).tile(
                    [P, 2 * (D + 1)], F32,
                    tag=("pv" if nt % 2 == 0 else "pps"), name=f"pv5w{nt}")
                    for nt in range(4)]

            q_t, k_t = qkT[jt]
            eps.setdefault(h, [])
            for mt in (range(4, NT) if h == 0 else range(NT)):
                sp = spsum.tile([P, N], F32, tag="sps", name="sps")
                for ch in range(NCH):
                    nc.tensor.matmul(
                        sp[:, ch * CHUNK:(ch + 1) * CHUNK],
                        lhsT=k_t[o:o + D, mt * P:(mt + 1) * P],
                        rhs=q_t[o:o + D, ch * CHUNK:(ch + 1) * CHUNK],
                        start=True, stop=True)
                ep = epool.tile([P, N], BF16, tag="exp", name="exp")
                nc.scalar.activation(out=ep[:], in_=sp[:], func=AFT.Exp,
                                     scale=SCALE)
                eps[h].append(ep)
                # spread filler evenly across the 96 S-tiles
                filler_target += FILLER_TOTAL / (H * NT - 4)
                popped = 0
                while fillers and filler_done < filler_target and popped < 2:
                    rows, _, fn = fillers.pop(0)
                    fn()
                    filler_done += rows
                    popped += 1
                if h == H - 1 and mt >= 2:
                    # weave head-10's PV accumulation (one open group per
                    # psum tile; head-11's group follows in the tail)
                    mtl = mt - 2
                    hx = 2 * jt
                    for wnt in range(4):
                        nc.tensor.matmul(
                            pvs5[wnt][:, 0:D + 1],
                            lhsT=eps[hx][mtl][:, wnt * P:(wnt + 1) * P],
                            rhs=v_aug[mtl][:, hx * (D + 1):
                                           (hx + 1) * (D + 1)],
                            start=(mtl == 0), stop=(mtl == NT - 1))

        # ---- post-window tail ----
        in_tail[0] = True
        # finish the woven pair-5 accumulators (last two mt slices), then
        # their normalize/transpose chains; token tiles 0-3 gate every ch0
        # output chunk
        for mtl in (NT - 2, NT - 1):
            hx = 2 * (CT - 1)
            for nt in range(4):
                nc.tensor.matmul(
                    pvs5[nt][:, 0:D + 1],
                    lhsT=eps[hx][mtl][:, nt * P:(nt + 1) * P],
                    rhs=v_aug[mtl][:, hx * (D + 1):(hx + 1) * (D + 1)],
                    start=(mtl == 0), stop=(mtl == NT - 1))
        for nt in range(4):
            hx = 2 * (CT - 1) + 1
            for mtl in range(NT):
                nc.tensor.matmul(
                    pvs5[nt][:, D + 1:2 * (D + 1)],
                    lhsT=eps[hx][mtl][:, nt * P:(nt + 1) * P],
                    rhs=v_aug[mtl][:, hx * (D + 1):(hx + 1) * (D + 1)],
                    start=(mtl == 0), stop=(mtl == NT - 1))
            pv_norm(CT - 1, nt, pvs5[nt])
        pv5 = [(lambda nt=nt: pv_group(CT - 1, nt, (eps[10], eps[11])))
               for nt in range(4, NT)]

        # ---- output projection (ch-major; pv5 hi-tiles interleaved) ----
        ci = 0
        for ch in range(NCH):
            for jt in range(CT):
                last = (ch == NCH - 1 and jt == CT - 1)
                ps = fps_psum(ci)
                for ct in range(CT):
                    nc.tensor.matmul(
                        ps[:], lhsT=pw_sb[ct][:, jt * P:(jt + 1) * P],
                        rhs=aoT_sb[ct][:, ch * CHUNK:(ch + 1) * CHUNK],
                        start=(ct == 0), stop=(ct == CT - 1))
                splits = 2 if last else 1
                sw = CHUNK // splits
                for s in range(splits):
                    ob = ypool.tile([P, sw], BF16, tag="ob", name="ob")
                    on_act = (ci + s) % 2 == 0
                    if on_act:
                        nc.scalar.activation(
                            out=ob[:], in_=ps[:, s * sw:(s + 1) * sw],
                            func=AFT.Identity, bias=pb_sb[:, jt:jt + 1])
                    else:
                        nc.vector.tensor_scalar_add(
                            ob[:], ps[:, s * sw:(s + 1) * sw],
                            pb_sb[:, jt:jt + 1])
                    dma_eng = nc.scalar if (on_act and ci >= 8) else nc.sync
                    dma_eng.dma_start(
                        out=yT[jt * P:(jt + 1) * P,
                               ch * CHUNK + s * sw:ch * CHUNK + (s + 1) * sw],
                        in_=ob[:])
                if pv5:
                    pv5.pop(0)()
                ci += 1

    nc.compile()
    return nc


def _get_nc(use_f32r=True):
    key = ("nc", use_f32r)
    if key not in _CACHE:
        _CACHE[key] = build_nc(use_f32r)
    return _CACHE[key]


def kernel(x, qkv_w, qkv_b, q_w, q_b, q_A, q_B, v_w, v_b, v_A, v_B,
           proj_w, proj_b, _trace=False, _use_f32r=True):
    x = np.ascontiguousarray(np.asarray(x, dtype=np.float32))
    B = x.shape[0]
    assert x.shape == (8, N, C)

    qkv_w = np.asarray(qkv_w, np.float32)
    qkv_b = np.asarray(qkv_b, np.float32)
    q_w = np.asarray(q_w, np.float32)
    q_b = np.asarray(q_b, np.float32)
    q_A = np.asarray(q_A, np.float32)
    q_B = np.asarray(q_B, np.float32)
    v_w = np.asarray(v_w, np.float32)
    v_b = np.asarray(v_b, np.float32)
    v_A = np.asarray(v_A, np.float32)
    v_B = np.asarray(v_B, np.float32)
    proj_w = np.asarray(proj_w, np.float32)
    proj_b = np.asarray(proj_b, np.float32)

    # exact algebraic folds (see module docstring)
    qw_eff = q_w + (q_B @ q_A) * (1.0 / R)
    vw_eff = v_w + (v_B @ v_A) * (1.0 / R)
    kw = qkv_w[C:2 * C]
    kb = qkv_b[C:2 * C]
    pb_eff = proj_b + proj_w @ v_b

    bf = ml_dtypes.bfloat16
    qwT_h = qw_eff.T.astype(bf)
    kwT_h = kw.T.astype(bf)
    wA_h = np.concatenate([qwT_h[:, 0:P], kwT_h[:, 0:P]], axis=1)
    qb_col = q_b.astype(bf).reshape(C, 1)
    kb_col = kb.astype(bf).reshape(C, 1)
    common = {
        "wR": np.ascontiguousarray(
            np.concatenate([qwT_h[:, 2 * P:C], kwT_h[:, 2 * P:C]], axis=1)),
        "vwT": np.ascontiguousarray(vw_eff.T.astype(bf)),
        "pwT": np.ascontiguousarray(proj_w.T.astype(bf)),
        "pb": np.ascontiguousarray(pb_eff.reshape(CT, P).T),
    }
    in_maps = [
        {"xA": np.ascontiguousarray(np.concatenate(
            [wA_h, qb_col, kb_col, x[i].T.astype(bf),
             qwT_h[:, P:2 * P], kwT_h[:, P:2 * P]], axis=1)), **common}
        for i in range(B)
    ]

    nc = _get_nc(_use_f32r)
    res = run_bass_kernel_spmd(nc, in_maps, list(range(B)), trace=_trace)

    out = np.empty((B, N, C), np.float32)
    for i in range(B):
        out[i] = res.results[i]["yT"].T.astype(np.float32)
    if _trace:
        return out, res
    return out
